# revision 27
# baseline (speedup 1.0000x reference)
"""2-layer GATv2 over 50k nodes / 1.6M edges on 8 trn2 NeuronCores.

Strategy (self-contained; shapes hardcoded for this problem):
  - Node-parallel dst sharding: nodes are degree-sorted and dealt round-robin
    to 8 cores (balanced slot counts); each core owns 6272 dst nodes.
  - Per-shard GEMMs only; per-shard xl tables are AllGathered on-device so
    each core can gather arbitrary source rows. Host->device traffic is just
    the core's own x shard (fp16), packed u16 slot tables, small weights and
    one constants row.
  - Per dst node, incoming edges live in up to D=64 "slots" (max degree 61);
    per-128-node-tile slot count Dt comes from the degree sort. Equal-Dt
    tiles are processed in chunks so every edge-phase vector op covers
    several tiles (few, large instructions).
  - att is folded into the weights on the host (u = att*z); leaky-relu
    logits use the identity 0.8*(sum_pos relu(u) - sum_neg relu(-u))
    + 0.2*sigma = 0.6*sigma + 0.4*(sum_pos|u| - sum_neg|u|), computed with
    two abs-reduces; messages are recovered from u via a 1/att columnwise
    multiply (exact up to fp rounding).
  - Gather of xl rows via per-slot indirect DMA (gpsimd SWDGE), bypass mode,
    pad indices clamped to NP-1 and masked to alpha=0 via an on-device
    idx>=NP -> -1e30 logit offset.
  - Layer-1 GEMMs compute [xl|xr] in one matmul pair per tile into a
    persistent SBUF tile; the xl half goes to DRAM in a single DMA for the
    AllGather. Layer-1 output h is transposed on PE into a resident fp16
    SBUF tile used directly as lhsT for the layer-2 GEMMs.
"""
import os
os.environ.setdefault("JAX_PLATFORMS", "cpu")
import sys
if "/opt/trn_rl_repo" not in sys.path:
    sys.path.insert(0, "/opt/trn_rl_repo")
import numpy as np
import concourse.bass as bass
import concourse.bacc as bacc
import concourse.mybir as mybir
import concourse.tile as tile
from concourse import bass_utils
from concourse.masks import make_identity

f32 = mybir.dt.float32
f16 = mybir.dt.float16
i32 = mybir.dt.int32
u16 = mybir.dt.uint16
AX = mybir.AxisListType
OP = mybir.AluOpType
AF = mybir.ActivationFunctionType

N = 50000
NCORES = 8
NP = 50176          # 8 * 6272, multiple of 1024
SH = NP // NCORES   # 6272 = 49 * 128
TPS = SH // 128     # 49 tiles per shard
F_IN = 256
H = 128
C = 64
DMAX = 64
NEG = 0.2
EPS = 1e-16
CAP = 192           # max slots (CH*Dt) per edge-phase chunk

VARIANT = "full"    # "full" | "nogather" | "noedge" (timing ablations)

LAST_RESULT = None
LAST_RUN_WALL = None
_PROGRAM_CACHE = {}


def ts(i, s):
    return slice(i * s, (i + 1) * s)


def ceil4(v):
    return max(4, (int(v) + 3) // 4 * 4)


def make_chunks(Dts, cap=CAP):
    """Group consecutive equal-Dt tiles into chunks of at most cap slots."""
    chunks = []
    t = 0
    while t < TPS:
        Dt = Dts[t]
        ch = 1
        while (t + ch < TPS and Dts[t + ch] == Dt and (ch + 1) * Dt <= cap):
            ch += 1
        chunks.append((t, ch, Dt))
        t += ch
    return chunks


def build_program(Dts, Fp1, Fp2):
    key = (tuple(Dts), Fp1, Fp2, VARIANT)
    if key in _PROGRAM_CACHE:
        return _PROGRAM_CACHE[key]
    TOTD = sum(Dts)
    cums = [0]
    for d in Dts:
        cums.append(cums[-1] + d)
    TW1, TW2 = H + 1, C + 1

    nc = bacc.Bacc("TRN2", target_bir_lowering=False, debug=False,
                   enable_asserts=False, num_devices=NCORES)

    xTo = nc.dram_tensor("xTo", [F_IN, SH], f16, kind="ExternalInput")
    # fused [wl|wr] weight tables
    w1 = nc.dram_tensor("w1", [F_IN, 2 * TW1], f16, kind="ExternalInput")
    w2 = nc.dram_tensor("w2", [H, 2 * TW2], f16, kind="ExternalInput")
    slotp = nc.dram_tensor("slotp", [128, TOTD], u16, kind="ExternalInput")
    cvec = nc.dram_tensor("cvec", [1, 2 * H + 2 * C], f32,
                          kind="ExternalInput")
    outc = nc.dram_tensor("outc", [SH, C], f16, kind="ExternalOutput")

    xl1t = nc.dram_tensor("xl1t", [SH, TW1], f16, kind="Internal")
    xl1f = nc.dram_tensor("xl1f", [NP, TW1], f16, kind="Internal",
                          addr_space="Shared")
    xl2t = nc.dram_tensor("xl2t", [SH, TW2], f16, kind="Internal")
    xl2f = nc.dram_tensor("xl2f", [NP, TW2], f16, kind="Internal",
                          addr_space="Shared")

    with tile.TileContext(nc) as tc:
        with tc.tile_pool(name="pers", bufs=1) as pers:
            # persistent SBUF residents (span all phases)
            x1_sb = pers.tile([128, TPS, 2, TW1], f16)   # [xl|xr] layer 1
            hT_sb = pers.tile([128, SH], f16)
            x2_sb = pers.tile([128, TPS, 2, TW2], f16)   # [xl|xr] layer 2
            slot_i = pers.tile([128, TOTD], i32)
            moff_sb = pers.tile([128, TOTD], f32)
            consts = pers.tile([128, 2 * H + 2 * C], f32)
            ident = pers.tile([128, 128], f32)
            w2_t = pers.tile([128, 2 * TW2], f16)

            make_identity(nc, ident[:])
            nc.sync.dma_start(out=w2_t[:], in_=w2.ap())

            rc1_t = consts[:, 0:H]
            cb1_t = consts[:, H:2 * H]
            rc2_t = consts[:, 2 * H:2 * H + C]
            cb2_t = consts[:, 2 * H + C:2 * H + 2 * C]

            # slot table: u16 load -> f32; mask from idx>=NP; clamp for the
            # gather (pads read row NP-1, masked to alpha=0 by moff)
            with (
                tc.tile_pool(name="pslot", bufs=1) as pslot,
                tc.tile_pool(name="pcp", bufs=1, space="PSUM") as pcp,
            ):
                crow = pslot.tile([1, 2 * H + 2 * C], f32)
                nc.sync.dma_start(out=crow[:], in_=cvec.ap())
                ones = pslot.tile([1, 128], f32)
                nc.vector.memset(ones[:], 1.0)
                cps = pcp.tile([128, 2 * H + 2 * C], f32)
                nc.tensor.matmul(out=cps[:], lhsT=ones[:], rhs=crow[:],
                                 start=True, stop=True)
                nc.scalar.copy(out=consts[:], in_=cps[:])

                slot_u = pslot.tile([128, TOTD], u16)
                nc.sync.dma_start(out=slot_u[:], in_=slotp.ap())
                slot_f = pslot.tile([128, TOTD], f32)
                nc.vector.tensor_copy(out=slot_f[:], in_=slot_u[:])
                nc.vector.tensor_scalar(out=moff_sb[:], in0=slot_f[:],
                                        scalar1=float(NP) - 0.5,
                                        scalar2=-1e30,
                                        op0=OP.is_ge, op1=OP.mult)
                nc.vector.tensor_scalar_min(slot_f[:], slot_f[:],
                                            float(NP - 1))
                nc.vector.tensor_copy(out=slot_i[:], in_=slot_f[:])

            # ---------------- Phase A: layer-1 GEMMs (own shard) ----------
            with (
                tc.tile_pool(name="paw", bufs=1) as pw,
                tc.tile_pool(name="pap", bufs=4, space="PSUM") as pp,
            ):
                w1_t = pw.tile([128, 2, 2 * TW1], f16)
                for k in range(2):
                    nc.sync.dma_start(out=w1_t[:, k, :],
                                      in_=w1.ap()[ts(k, 128), :])
                x_sb = pw.tile([128, 2, SH], f16)
                for k in range(2):
                    nc.sync.dma_start(out=x_sb[:, k, :],
                                      in_=xTo.ap()[ts(k, 128), :])
                for t in range(TPS):
                    ps_t = pp.tile([128, 2 * TW1], f32, tag="ps")
                    for k in range(2):
                        nc.tensor.matmul(out=ps_t[:],
                                         lhsT=x_sb[:, k, ts(t, 128)],
                                         rhs=w1_t[:, k, :],
                                         start=(k == 0), stop=(k == 1))
                    nc.scalar.copy(
                        out=x1_sb[:, t, :, :].rearrange("p a w -> p (a w)"),
                        in_=ps_t[:])
                nc.sync.dma_start(
                    out=xl1t.ap().rearrange("(t p) w -> p t w", p=128),
                    in_=x1_sb[:, :, 0, :])

            # ---------------- AllGather layer-1 xl table ------------------
            nc.gpsimd.collective_compute(
                "AllGather", OP.bypass,
                replica_groups=[list(range(NCORES))],
                ins=[xl1t.ap()], outs=[xl1f.ap()])

            # ---------------- layer-1 edge phase --------------------------
            if VARIANT != "noedge":
                edge_phase(nc, tc, Dts, cums, Fp1, H, xl1f, x1_sb, slot_i,
                           moff_sb, rc1_t, cb1_t, relu=True, out_dram=None,
                           hT_sb=hT_sb, ident=ident)
            else:
                nc.vector.memset(hT_sb[:], 0.0)

            # ---------------- Phase C: layer-2 GEMMs (own shard) ----------
            with tc.tile_pool(name="pdp", bufs=4, space="PSUM") as pp2:
                for t in range(TPS):
                    ps2_t = pp2.tile([128, 2 * TW2], f32, tag="ps2")
                    nc.tensor.matmul(out=ps2_t[:], lhsT=hT_sb[:, ts(t, 128)],
                                     rhs=w2_t[:], start=True, stop=True)
                    nc.scalar.copy(
                        out=x2_sb[:, t, :, :].rearrange("p a w -> p (a w)"),
                        in_=ps2_t[:])
                nc.sync.dma_start(
                    out=xl2t.ap().rearrange("(t p) w -> p t w", p=128),
                    in_=x2_sb[:, :, 0, :])

            # ---------------- AllGather layer-2 xl table ------------------
            nc.gpsimd.collective_compute(
                "AllGather", OP.bypass,
                replica_groups=[list(range(NCORES))],
                ins=[xl2t.ap()], outs=[xl2f.ap()])

            # ---------------- layer-2 edge phase --------------------------
            if VARIANT != "noedge":
                edge_phase(nc, tc, Dts, cums, Fp2, C, xl2f, x2_sb, slot_i,
                           moff_sb, rc2_t, cb2_t, relu=False, out_dram=outc,
                           hT_sb=None, ident=None)
            else:
                with tc.tile_pool(name="pdum", bufs=2) as pdum:
                    dt_ = pdum.tile([128, TPS, C], f16)
                    nc.vector.tensor_copy(out=dt_[:], in_=x2_sb[:, :, 1, 0:C])
                    nc.sync.dma_start(
                        out=outc.ap().rearrange("(t p) w -> p t w", p=128),
                        in_=dt_[:])

    nc.compile()
    _PROGRAM_CACHE[key] = nc
    return nc


def edge_phase(nc, tc, Dts, cums, Fp, F, xlf, xlr_sb, slot_i, moff_sb,
               rc_t, cb_t, relu, out_dram, hT_sb, ident):
    TW = F + 1   # table width: F features + q (= row-sum) column
    with (
        tc.tile_pool(name=f"pz{F}", bufs=2) as pz,
        tc.tile_pool(name=f"pm{F}", bufs=3) as psm,
        tc.tile_pool(name=f"po{F}", bufs=2) as pout,
        tc.tile_pool(name=f"pp{F}", bufs=2, space="PSUM") as pps,
    ):
        for (t0, CH, Dt) in make_chunks(Dts):
            S = CH * Dt
            cu = cums[t0]
            off_t = moff_sb[:, cu:cu + S]
            xr_c = xlr_sb[:, t0:t0 + CH, 1, :]

            # z = gathered xl rows (bypass; pad idx clamped to NP-1), then
            # u = z + xr via in-place broadcast add.
            z_t = pz.tile([128, CH, Dt, TW], f16, tag="z")
            zf = z_t[:].rearrange("p c d w -> p (c d) w")
            if VARIANT == "nogather":
                nc.vector.tensor_copy(
                    out=z_t[:],
                    in_=xr_c[:, :, None, :].to_broadcast([128, CH, Dt, TW]))
            else:
                for c in range(CH):
                    cuc = cums[t0 + c]
                    for d in range(Dt):
                        nc.gpsimd.indirect_dma_start(
                            out=z_t[:, c, d, :], out_offset=None,
                            in_=xlf.ap(),
                            in_offset=bass.IndirectOffsetOnAxis(
                                ap=slot_i[:, cuc + d:cuc + d + 1], axis=0),
                            bounds_check=None, compute_op=OP.bypass)
            nc.vector.tensor_tensor(
                out=z_t[:], in0=z_t[:],
                in1=xr_c[:, :, None, :].to_broadcast([128, CH, Dt, TW]),
                op=OP.add)

            # logits. With u = xl+xr (in z) and the q column sigma = sum_f u:
            #   0.8*(sum_pos relu(u) - sum_neg relu(-u)) + 0.2*sigma
            #     = 0.6*sigma + 0.4*(sum_pos |u| - sum_neg |u|)
            # so two abs-reduces replace the ACT relus entirely.
            e_t = psm.tile([128, CH, Dt], f32, tag="e")
            ef = e_t[:].rearrange("p c d -> p (c d)")
            ep_t = psm.tile([128, S], f32, tag="ep")
            nc.vector.tensor_reduce(out=ep_t[:], in_=zf[:, :, 0:Fp],
                                    axis=AX.X, op=OP.add,
                                    apply_absolute_value=True)
            en_t = psm.tile([128, S], f32, tag="en")
            nc.vector.tensor_reduce(out=en_t[:], in_=zf[:, :, Fp:F],
                                    axis=AX.X, op=OP.add,
                                    apply_absolute_value=True)
            nc.vector.scalar_tensor_tensor(out=ef, in0=en_t[:],
                                           scalar=-1.0, in1=ep_t[:],
                                           op0=OP.mult, op1=OP.add)
            nc.vector.scalar_tensor_tensor(out=ef, in0=zf[:, :, F],
                                           scalar=1.5, in1=ef,
                                           op0=OP.mult, op1=OP.add)
            nc.vector.scalar_tensor_tensor(out=ef, in0=ef,
                                           scalar=0.4, in1=off_t,
                                           op0=OP.mult, op1=OP.add)
            # softmax over slots of each dst node (c-row)
            mneg_t = psm.tile([128, CH], f32, tag="mneg")
            nc.vector.tensor_reduce(out=mneg_t[:], in_=e_t[:], axis=AX.X,
                                    op=OP.max, negate=True)
            nc.vector.tensor_scalar_min(mneg_t[:], mneg_t[:], 1e29)
            nc.vector.tensor_tensor(
                out=e_t[:], in0=e_t[:],
                in1=mneg_t[:, :, None].to_broadcast([128, CH, Dt]),
                op=OP.add)
            a_t = psm.tile([128, CH, Dt], f32, tag="a")
            nc.scalar.activation(out=a_t[:].rearrange("p c d -> p (c d)"),
                                 in_=ef, func=AF.Exp)
            s_t = psm.tile([128, CH], f32, tag="s")
            nc.vector.tensor_reduce(out=s_t[:], in_=a_t[:], axis=AX.X,
                                    op=OP.add)
            nc.vector.tensor_scalar_add(s_t[:], s_t[:], EPS)
            r_t = psm.tile([128, CH], f32, tag="r")
            nc.vector.reciprocal(out=r_t[:], in_=s_t[:])
            al_t = psm.tile([128, CH, Dt], f32, tag="al")
            nc.vector.tensor_tensor(
                out=al_t[:], in0=a_t[:],
                in1=r_t[:, :, None].to_broadcast([128, CH, Dt]), op=OP.mult)

            # message aggregation: msg = sum_d alpha_d * u_d - (sum alpha)*xr
            # (z holds u = xr+g). In-place alpha multiply + one reduce over
            # the slot axis via a transposed AP view.
            nc.vector.tensor_tensor(
                out=z_t[:, :, :, 0:F], in0=z_t[:, :, :, 0:F],
                in1=al_t[:, :, :, None].to_broadcast([128, CH, Dt, F]),
                op=OP.mult)
            acc_t = pout.tile([128, CH, F], f32, tag="acc")
            nc.vector.tensor_reduce(
                out=acc_t[:], in_=z_t[:, :, :, 0:F].transpose([0, 1, 3, 2]),
                axis=AX.X, op=OP.add)
            saneg_t = psm.tile([128, CH], f32, tag="saneg")
            nc.vector.tensor_reduce(out=saneg_t[:], in_=al_t[:],
                                    axis=AX.X, op=OP.add, negate=True)
            hh_t = pout.tile([128, CH, F], f32, tag="hh")
            nc.vector.tensor_tensor(
                out=hh_t[:], in0=xr_c[:, :, 0:F],
                in1=saneg_t[:, :, None].to_broadcast([128, CH, F]),
                op=OP.mult)
            nc.vector.tensor_tensor(out=hh_t[:], in0=hh_t[:], in1=acc_t[:],
                                    op=OP.add)
            nc.vector.tensor_tensor(
                out=hh_t[:], in0=hh_t[:],
                in1=rc_t[:, None, :].to_broadcast([128, CH, F]), op=OP.mult)
            nc.vector.tensor_tensor(
                out=hh_t[:], in0=hh_t[:],
                in1=cb_t[:, None, :].to_broadcast([128, CH, F]), op=OP.add)
            if relu:
                nc.vector.tensor_scalar_max(hh_t[:], hh_t[:], 0.0)
                for c in range(CH):
                    pt_t = pps.tile([128, 128], f32, tag="pt")
                    nc.tensor.transpose(out=pt_t[:], in_=hh_t[:, c, :],
                                        identity=ident[:])
                    nc.scalar.copy(out=hT_sb[:, ts(t0 + c, 128)], in_=pt_t[:])
            else:
                ho_t = pout.tile([128, CH, F], f16, tag="ho")
                nc.vector.tensor_copy(out=ho_t[:], in_=hh_t[:])
                nc.sync.dma_start(
                    out=out_dram.ap()[t0 * 128:(t0 + CH) * 128, :].rearrange(
                        "(c p) w -> p c w", p=128),
                    in_=ho_t[:])


def prepare_host(x, edge_index, Wl1, Wr1, att1, b1, Wl2, Wr2, att2, b2):
    src = np.asarray(edge_index[0], dtype=np.int64)
    dst = np.asarray(edge_index[1], dtype=np.int64)
    x = np.asarray(x, dtype=np.float32)

    deg = np.bincount(dst, minlength=NP).astype(np.int64)
    assert deg.max() <= DMAX, f"max degree {deg.max()} > {DMAX}"
    order = np.argsort(-deg, kind="stable")
    q = np.arange(NP)
    new_of = np.empty(NP, dtype=np.int64)
    new_of[order] = (q % NCORES) * SH + q // NCORES
    glob_of_new = np.empty(NP, dtype=np.int64)
    glob_of_new[new_of] = np.arange(NP)

    # slot tables (values are NEW ids; rows ordered by NEW id)
    eorder = np.argsort(dst, kind="stable")
    s_src = src[eorder]
    s_dst = dst[eorder]
    starts = np.zeros(NP, dtype=np.int64)
    starts[1:] = np.cumsum(deg)[:-1]
    pos = np.arange(len(s_dst)) - starts[s_dst]
    # pads carry idx=NP: clamped to NP-1 for the gather, masked by the
    # on-device idx>=NP -> -1e30 logit offset
    slot_g = np.full((NP, DMAX), NP, dtype=np.int32)
    slot_g[s_dst, pos] = new_of[s_src].astype(np.int32)
    slot_new = slot_g[glob_of_new]

    deg_sorted = deg[order]
    Dts = tuple(ceil4(max(deg_sorted[1024 * t], 1)) for t in range(TPS))

    att1 = np.asarray(att1, np.float32)
    att2 = np.asarray(att2, np.float32)
    assert np.abs(att1).min() > 1e-8 and np.abs(att2).min() > 1e-8
    p1 = np.argsort(att1 < 0, kind="stable")
    Fp1 = int((att1 >= 0).sum())
    p2 = np.argsort(att2 < 0, kind="stable")
    Fp2 = int((att2 >= 0).sum())
    # fold att into weight columns, sign-permute, and append a row-sum
    # column (the q/sigma channel: sum_f u = x @ wsum)
    def fold(W, att, perm, rowperm=None):
        Wa = (np.asarray(W, np.float32) * att)
        if rowperm is not None:
            Wa = Wa[rowperm, :]
        Wp = Wa[:, perm]
        return np.concatenate([Wp, Wp.sum(1, keepdims=True)], axis=1)

    Wl1a = fold(Wl1, att1, p1)
    Wr1a = fold(Wr1, att1, p1)
    Wl2a = fold(Wl2, att2, p2, rowperm=p1)
    Wr2a = fold(Wr2, att2, p2, rowperm=p1)
    w1c = np.ascontiguousarray(
        np.concatenate([Wl1a, Wr1a], axis=1), np.float16)
    w2c = np.ascontiguousarray(
        np.concatenate([Wl2a, Wr2a], axis=1), np.float16)
    rc1_row = (1.0 / att1[p1]).astype(np.float32)
    rc2_row = (1.0 / att2[p2]).astype(np.float32)
    b1_row = np.asarray(b1, np.float32)[p1]
    b2_row = np.asarray(b2, np.float32)[p2]
    cvec = np.concatenate([rc1_row, b1_row, rc2_row, b2_row])[None, :]
    cvec = np.ascontiguousarray(cvec, np.float32)

    xp = np.zeros((NP, F_IN), np.float32)
    xp[:N] = x
    xT_perm = xp[glob_of_new].T.astype(np.float16)

    common = dict(w1=w1c, w2=w2c, cvec=cvec)
    in_maps = []
    for c in range(NCORES):
        m = dict(common)
        m["xTo"] = np.ascontiguousarray(xT_perm[:, ts(c, SH)])
        sl = slot_new[ts(c, SH)]
        m["slotp"] = np.ascontiguousarray(np.concatenate(
            [sl[ts(t, 128), 0:Dts[t]] for t in range(TPS)],
            axis=1).astype(np.uint16))
        in_maps.append(m)
    return in_maps, Dts, Fp1, Fp2, glob_of_new, p2


def kernel(**inputs):
    global LAST_RESULT, LAST_RUN_WALL
    import time as _time
    in_maps, Dts, Fp1, Fp2, glob_of_new, p2 = prepare_host(**inputs)
    nc = build_program(Dts, Fp1, Fp2)
    _t0 = _time.time()
    res = bass_utils.run_bass_kernel_spmd(nc, in_maps,
                                          core_ids=list(range(NCORES)))
    LAST_RUN_WALL = _time.time() - _t0
    LAST_RESULT = res
    out_new = np.concatenate([res.results[c]["outc"] for c in range(NCORES)],
                             axis=0).astype(np.float32)
    out_glob = np.empty((NP, C), np.float32)
    out_glob[glob_of_new] = out_new
    return np.ascontiguousarray(out_glob[:N][:, np.argsort(p2)])


# revision 32
# speedup vs baseline: 2.4723x; 2.4723x over previous
"""2-layer GATv2 over 50k nodes / 1.6M edges on 8 trn2 NeuronCores.

Strategy (self-contained; shapes hardcoded for this problem):
  - Node-parallel dst sharding: nodes are degree-sorted and dealt round-robin
    to 8 cores (balanced slot counts); each core owns 6272 dst nodes.
  - Per-shard GEMMs only; per-shard xl tables are AllGathered on-device so
    each core can gather arbitrary source rows. Host->device traffic is just
    the core's own x shard (fp16), packed u16 slot tables, small weights and
    one constants row.
  - Per dst node, incoming edges live in up to D=64 "slots" (max degree 61);
    per-128-node-tile slot count Dt comes from the degree sort. Equal-Dt
    tiles are processed in chunks so every edge-phase vector op covers
    several tiles (few, large instructions).
  - att is folded into the weights on the host (u = att*z); leaky-relu
    logits use the identity 0.8*(sum_pos relu(u) - sum_neg relu(-u))
    + 0.2*sigma = 0.6*sigma + 0.4*(sum_pos|u| - sum_neg|u|), computed with
    two abs-reduces; messages are recovered from u via a 1/att columnwise
    multiply (exact up to fp rounding).
  - Gather of xl rows via per-slot indirect DMA (gpsimd SWDGE), bypass mode,
    pad indices clamped to NP-1 and masked to alpha=0 via an on-device
    idx>=NP -> -1e30 logit offset.
  - Layer-1 GEMMs compute [xl|xr] in one matmul pair per tile into a
    persistent SBUF tile; the xl half goes to DRAM in a single DMA for the
    AllGather. Layer-1 output h is transposed on PE into a resident fp16
    SBUF tile used directly as lhsT for the layer-2 GEMMs.
"""
import os
os.environ.setdefault("JAX_PLATFORMS", "cpu")
import sys
if "/opt/trn_rl_repo" not in sys.path:
    sys.path.insert(0, "/opt/trn_rl_repo")
import numpy as np
import concourse.bass as bass
import concourse.bacc as bacc
import concourse.mybir as mybir
import concourse.tile as tile
from concourse import bass_utils
from concourse.masks import make_identity

f32 = mybir.dt.float32
f16 = mybir.dt.float16
i32 = mybir.dt.int32
u16 = mybir.dt.uint16
AX = mybir.AxisListType
OP = mybir.AluOpType
AF = mybir.ActivationFunctionType

N = 50000
NCORES = 8
NP = 50176          # 8 * 6272, multiple of 1024
SH = NP // NCORES   # 6272 = 49 * 128
TPS = SH // 128     # 49 tiles per shard
F_IN = 256
H = 128
C = 64
DMAX = 64
NEG = 0.2
EPS = 1e-16
CAP = 192           # max slots (CH*Dt) per edge-phase chunk

VARIANT = "full"    # "full" | "nogather" | "noedge" (timing ablations)

LAST_RESULT = None
LAST_RUN_WALL = None
_PROGRAM_CACHE = {}


def ts(i, s):
    return slice(i * s, (i + 1) * s)


def ceil4(v):
    return max(4, (int(v) + 3) // 4 * 4)


def make_chunks(Dts, cap=CAP):
    """Group consecutive equal-Dt tiles into chunks of at most cap slots."""
    chunks = []
    t = 0
    while t < TPS:
        Dt = Dts[t]
        ch = 1
        while (t + ch < TPS and Dts[t + ch] == Dt and (ch + 1) * Dt <= cap):
            ch += 1
        chunks.append((t, ch, Dt))
        t += ch
    return chunks


def build_program(Dts, Fp1, Fp2):
    key = (tuple(Dts), Fp1, Fp2, VARIANT)
    if key in _PROGRAM_CACHE:
        return _PROGRAM_CACHE[key]
    TOTD = sum(Dts)
    cums = [0]
    for d in Dts:
        cums.append(cums[-1] + d)
    TW1, TW2 = H + 1, C + 1

    nc = bacc.Bacc("TRN2", target_bir_lowering=False, debug=False,
                   enable_asserts=False, num_devices=NCORES)

    # all inputs packed into one u16 blob (fewer PJRT operands):
    # [xTo f16 | w1 f16 | w2 f16 | cvec f16 | slotp u16]
    LX = F_IN * SH
    LW1 = F_IN * 2 * TW1
    LW2 = H * 2 * TW2
    LCV = 2 * H + 2 * C
    LSL = 128 * TOTD
    o_w1 = LX
    o_w2 = o_w1 + LW1
    o_cv = o_w2 + LW2
    o_sl = o_cv + LCV
    TOTB = o_sl + LSL
    blob = nc.dram_tensor("blob", [TOTB], u16, kind="ExternalInput")
    bap = blob.ap()
    xTo_ap = bap[0:LX].rearrange("(r c) -> r c", r=F_IN).bitcast(f16)
    w1_ap = bap[o_w1:o_w1 + LW1].rearrange("(r c) -> r c",
                                           r=F_IN).bitcast(f16)
    w2_ap = bap[o_w2:o_w2 + LW2].rearrange("(r c) -> r c", r=H).bitcast(f16)
    cv_ap = bap[o_cv:o_cv + LCV].rearrange("(r c) -> r c", r=1).bitcast(f16)
    sl_ap = bap[o_sl:o_sl + LSL].rearrange("(r c) -> r c", r=128)
    outc = nc.dram_tensor("outc", [SH, C], f16, kind="ExternalOutput")

    xl1t = nc.dram_tensor("xl1t", [SH, TW1], f16, kind="Internal")
    xl1f = nc.dram_tensor("xl1f", [NP, TW1], f16, kind="Internal",
                          addr_space="Shared")
    xl2t = nc.dram_tensor("xl2t", [SH, TW2], f16, kind="Internal")
    xl2f = nc.dram_tensor("xl2f", [NP, TW2], f16, kind="Internal",
                          addr_space="Shared")

    with tile.TileContext(nc) as tc:
        with tc.tile_pool(name="pers", bufs=1) as pers:
            # persistent SBUF residents (span all phases)
            x1_sb = pers.tile([128, TPS, 2, TW1], f16)   # [xl|xr] layer 1
            hT_sb = pers.tile([128, SH], f16)
            x2_sb = pers.tile([128, TPS, 2, TW2], f16)   # [xl|xr] layer 2
            slot_i = pers.tile([128, TOTD], i32)
            moff_sb = pers.tile([128, TOTD], f32)
            consts = pers.tile([128, 2 * H + 2 * C], f32)
            ident = pers.tile([128, 128], f32)
            w2_t = pers.tile([128, 2 * TW2], f16)

            make_identity(nc, ident[:])
            nc.sync.dma_start(out=w2_t[:], in_=w2_ap)

            rc1_t = consts[:, 0:H]
            cb1_t = consts[:, H:2 * H]
            rc2_t = consts[:, 2 * H:2 * H + C]
            cb2_t = consts[:, 2 * H + C:2 * H + 2 * C]

            # slot table: u16 load -> f32; mask from idx>=NP; clamp for the
            # gather (pads read row NP-1, masked to alpha=0 by moff)
            with (
                tc.tile_pool(name="pslot", bufs=1) as pslot,
                tc.tile_pool(name="pcp", bufs=1, space="PSUM") as pcp,
            ):
                crow = pslot.tile([1, 2 * H + 2 * C], f16)
                nc.sync.dma_start(out=crow[:], in_=cv_ap)
                ones = pslot.tile([1, 128], f16)
                nc.vector.memset(ones[:], 1.0)
                cps = pcp.tile([128, 2 * H + 2 * C], f32)
                nc.tensor.matmul(out=cps[:], lhsT=ones[:], rhs=crow[:],
                                 start=True, stop=True)
                nc.scalar.copy(out=consts[:], in_=cps[:])

                slot_u = pslot.tile([128, TOTD], u16)
                nc.sync.dma_start(out=slot_u[:], in_=sl_ap)
                slot_f = pslot.tile([128, TOTD], f32)
                nc.vector.tensor_copy(out=slot_f[:], in_=slot_u[:])
                nc.vector.tensor_scalar(out=moff_sb[:], in0=slot_f[:],
                                        scalar1=float(NP) - 0.5,
                                        scalar2=-1e30,
                                        op0=OP.is_ge, op1=OP.mult)
                nc.vector.tensor_scalar_min(slot_f[:], slot_f[:],
                                            float(NP - 1))
                nc.vector.tensor_copy(out=slot_i[:], in_=slot_f[:])

            # ---------------- Phase A: layer-1 GEMMs (own shard) ----------
            with (
                tc.tile_pool(name="paw", bufs=1) as pw,
                tc.tile_pool(name="pap", bufs=4, space="PSUM") as pp,
            ):
                w1_t = pw.tile([128, 2, 2 * TW1], f16)
                for k in range(2):
                    nc.sync.dma_start(out=w1_t[:, k, :],
                                      in_=w1_ap[ts(k, 128), :])
                x_sb = pw.tile([128, 2, SH], f16)
                for k in range(2):
                    nc.sync.dma_start(out=x_sb[:, k, :],
                                      in_=xTo_ap[ts(k, 128), :])
                for t in range(TPS):
                    ps_t = pp.tile([128, 2 * TW1], f32, tag="ps")
                    for k in range(2):
                        nc.tensor.matmul(out=ps_t[:],
                                         lhsT=x_sb[:, k, ts(t, 128)],
                                         rhs=w1_t[:, k, :],
                                         start=(k == 0), stop=(k == 1))
                    nc.scalar.copy(
                        out=x1_sb[:, t, :, :].rearrange("p a w -> p (a w)"),
                        in_=ps_t[:])
                nc.sync.dma_start(
                    out=xl1t.ap().rearrange("(t p) w -> p t w", p=128),
                    in_=x1_sb[:, :, 0, :])

            # ---------------- AllGather layer-1 xl table ------------------
            nc.gpsimd.collective_compute(
                "AllGather", OP.bypass,
                replica_groups=[list(range(NCORES))],
                ins=[xl1t.ap()], outs=[xl1f.ap()])

            # ---------------- layer-1 edge phase --------------------------
            if VARIANT != "noedge":
                edge_phase(nc, tc, Dts, cums, Fp1, H, xl1f, x1_sb, slot_i,
                           moff_sb, rc1_t, cb1_t, relu=True, out_dram=None,
                           hT_sb=hT_sb, ident=ident)
            else:
                nc.vector.memset(hT_sb[:], 0.0)

            # ---------------- Phase C: layer-2 GEMMs (own shard) ----------
            with tc.tile_pool(name="pdp", bufs=4, space="PSUM") as pp2:
                for t in range(TPS):
                    ps2_t = pp2.tile([128, 2 * TW2], f32, tag="ps2")
                    nc.tensor.matmul(out=ps2_t[:], lhsT=hT_sb[:, ts(t, 128)],
                                     rhs=w2_t[:], start=True, stop=True)
                    nc.scalar.copy(
                        out=x2_sb[:, t, :, :].rearrange("p a w -> p (a w)"),
                        in_=ps2_t[:])
                nc.sync.dma_start(
                    out=xl2t.ap().rearrange("(t p) w -> p t w", p=128),
                    in_=x2_sb[:, :, 0, :])

            # ---------------- AllGather layer-2 xl table ------------------
            nc.gpsimd.collective_compute(
                "AllGather", OP.bypass,
                replica_groups=[list(range(NCORES))],
                ins=[xl2t.ap()], outs=[xl2f.ap()])

            # ---------------- layer-2 edge phase --------------------------
            if VARIANT != "noedge":
                edge_phase(nc, tc, Dts, cums, Fp2, C, xl2f, x2_sb, slot_i,
                           moff_sb, rc2_t, cb2_t, relu=False, out_dram=outc,
                           hT_sb=None, ident=None)
            else:
                with tc.tile_pool(name="pdum", bufs=2) as pdum:
                    dt_ = pdum.tile([128, TPS, C], f16)
                    nc.vector.tensor_copy(out=dt_[:], in_=x2_sb[:, :, 1, 0:C])
                    nc.sync.dma_start(
                        out=outc.ap().rearrange("(t p) w -> p t w", p=128),
                        in_=dt_[:])

    nc.compile()
    _PROGRAM_CACHE[key] = nc
    return nc


def edge_phase(nc, tc, Dts, cums, Fp, F, xlf, xlr_sb, slot_i, moff_sb,
               rc_t, cb_t, relu, out_dram, hT_sb, ident):
    TW = F + 1   # table width: F features + q (= row-sum) column
    with (
        tc.tile_pool(name=f"pz{F}", bufs=2) as pz,
        tc.tile_pool(name=f"pm{F}", bufs=3) as psm,
        tc.tile_pool(name=f"po{F}", bufs=2) as pout,
        tc.tile_pool(name=f"pp{F}", bufs=2, space="PSUM") as pps,
    ):
        for (t0, CH, Dt) in make_chunks(Dts):
            S = CH * Dt
            cu = cums[t0]
            off_t = moff_sb[:, cu:cu + S]
            xr_c = xlr_sb[:, t0:t0 + CH, 1, :]

            # z = gathered xl rows (bypass; pad idx clamped to NP-1), then
            # u = z + xr via in-place broadcast add.
            z_t = pz.tile([128, CH, Dt, TW], f16, tag="z")
            zf = z_t[:].rearrange("p c d w -> p (c d) w")
            if VARIANT == "nogather":
                nc.vector.tensor_copy(
                    out=z_t[:],
                    in_=xr_c[:, :, None, :].to_broadcast([128, CH, Dt, TW]))
            else:
                for c in range(CH):
                    cuc = cums[t0 + c]
                    for d in range(Dt):
                        nc.gpsimd.indirect_dma_start(
                            out=z_t[:, c, d, :], out_offset=None,
                            in_=xlf.ap(),
                            in_offset=bass.IndirectOffsetOnAxis(
                                ap=slot_i[:, cuc + d:cuc + d + 1], axis=0),
                            bounds_check=None, compute_op=OP.bypass)
            nc.vector.tensor_tensor(
                out=z_t[:], in0=z_t[:],
                in1=xr_c[:, :, None, :].to_broadcast([128, CH, Dt, TW]),
                op=OP.add)

            # logits. With u = xl+xr (in z) and the q column sigma = sum_f u:
            #   0.8*(sum_pos relu(u) - sum_neg relu(-u)) + 0.2*sigma
            #     = 0.6*sigma + 0.4*(sum_pos |u| - sum_neg |u|)
            # so two abs-reduces replace the ACT relus entirely.
            e_t = psm.tile([128, CH, Dt], f32, tag="e")
            ef = e_t[:].rearrange("p c d -> p (c d)")
            ep_t = psm.tile([128, S], f32, tag="ep")
            nc.vector.tensor_reduce(out=ep_t[:], in_=zf[:, :, 0:Fp],
                                    axis=AX.X, op=OP.add,
                                    apply_absolute_value=True)
            en_t = psm.tile([128, S], f32, tag="en")
            nc.vector.tensor_reduce(out=en_t[:], in_=zf[:, :, Fp:F],
                                    axis=AX.X, op=OP.add,
                                    apply_absolute_value=True)
            nc.vector.scalar_tensor_tensor(out=ef, in0=en_t[:],
                                           scalar=-1.0, in1=ep_t[:],
                                           op0=OP.mult, op1=OP.add)
            nc.vector.scalar_tensor_tensor(out=ef, in0=zf[:, :, F],
                                           scalar=1.5, in1=ef,
                                           op0=OP.mult, op1=OP.add)
            nc.vector.scalar_tensor_tensor(out=ef, in0=ef,
                                           scalar=0.4, in1=off_t,
                                           op0=OP.mult, op1=OP.add)
            # softmax over slots of each dst node (c-row)
            mneg_t = psm.tile([128, CH], f32, tag="mneg")
            nc.vector.tensor_reduce(out=mneg_t[:], in_=e_t[:], axis=AX.X,
                                    op=OP.max, negate=True)
            nc.vector.tensor_scalar_min(mneg_t[:], mneg_t[:], 1e29)
            nc.vector.tensor_tensor(
                out=e_t[:], in0=e_t[:],
                in1=mneg_t[:, :, None].to_broadcast([128, CH, Dt]),
                op=OP.add)
            a_t = psm.tile([128, CH, Dt], f32, tag="a")
            nc.scalar.activation(out=a_t[:].rearrange("p c d -> p (c d)"),
                                 in_=ef, func=AF.Exp)
            s_t = psm.tile([128, CH], f32, tag="s")
            nc.vector.tensor_reduce(out=s_t[:], in_=a_t[:], axis=AX.X,
                                    op=OP.add)
            nc.vector.tensor_scalar_add(s_t[:], s_t[:], EPS)
            r_t = psm.tile([128, CH], f32, tag="r")
            nc.vector.reciprocal(out=r_t[:], in_=s_t[:])
            al_t = psm.tile([128, CH, Dt], f32, tag="al")
            nc.vector.tensor_tensor(
                out=al_t[:], in0=a_t[:],
                in1=r_t[:, :, None].to_broadcast([128, CH, Dt]), op=OP.mult)

            # message aggregation: msg = sum_d alpha_d * u_d - (sum alpha)*xr
            # (z holds u = xr+g). In-place alpha multiply + one reduce over
            # the slot axis via a transposed AP view.
            nc.vector.tensor_tensor(
                out=z_t[:, :, :, 0:F], in0=z_t[:, :, :, 0:F],
                in1=al_t[:, :, :, None].to_broadcast([128, CH, Dt, F]),
                op=OP.mult)
            acc_t = pout.tile([128, CH, F], f32, tag="acc")
            nc.vector.tensor_reduce(
                out=acc_t[:], in_=z_t[:, :, :, 0:F].transpose([0, 1, 3, 2]),
                axis=AX.X, op=OP.add)
            saneg_t = psm.tile([128, CH], f32, tag="saneg")
            nc.vector.tensor_reduce(out=saneg_t[:], in_=al_t[:],
                                    axis=AX.X, op=OP.add, negate=True)
            hh_t = pout.tile([128, CH, F], f32, tag="hh")
            nc.vector.tensor_tensor(
                out=hh_t[:], in0=xr_c[:, :, 0:F],
                in1=saneg_t[:, :, None].to_broadcast([128, CH, F]),
                op=OP.mult)
            nc.vector.tensor_tensor(out=hh_t[:], in0=hh_t[:], in1=acc_t[:],
                                    op=OP.add)
            nc.vector.tensor_tensor(
                out=hh_t[:], in0=hh_t[:],
                in1=rc_t[:, None, :].to_broadcast([128, CH, F]), op=OP.mult)
            nc.vector.tensor_tensor(
                out=hh_t[:], in0=hh_t[:],
                in1=cb_t[:, None, :].to_broadcast([128, CH, F]), op=OP.add)
            if relu:
                nc.vector.tensor_scalar_max(hh_t[:], hh_t[:], 0.0)
                for c in range(CH):
                    pt_t = pps.tile([128, 128], f32, tag="pt")
                    nc.tensor.transpose(out=pt_t[:], in_=hh_t[:, c, :],
                                        identity=ident[:])
                    nc.scalar.copy(out=hT_sb[:, ts(t0 + c, 128)], in_=pt_t[:])
            else:
                ho_t = pout.tile([128, CH, F], f16, tag="ho")
                nc.vector.tensor_copy(out=ho_t[:], in_=hh_t[:])
                nc.sync.dma_start(
                    out=out_dram.ap()[t0 * 128:(t0 + CH) * 128, :].rearrange(
                        "(c p) w -> p c w", p=128),
                    in_=ho_t[:])


def prepare_host(x, edge_index, Wl1, Wr1, att1, b1, Wl2, Wr2, att2, b2):
    src = np.asarray(edge_index[0], dtype=np.int64)
    dst = np.asarray(edge_index[1], dtype=np.int64)
    x = np.asarray(x, dtype=np.float32)

    deg = np.bincount(dst, minlength=NP).astype(np.int64)
    assert deg.max() <= DMAX, f"max degree {deg.max()} > {DMAX}"
    order = np.argsort(-deg, kind="stable")
    q = np.arange(NP)
    new_of = np.empty(NP, dtype=np.int64)
    new_of[order] = (q % NCORES) * SH + q // NCORES
    glob_of_new = np.empty(NP, dtype=np.int64)
    glob_of_new[new_of] = np.arange(NP)

    # slot tables (values are NEW ids; rows ordered by NEW id)
    eorder = np.argsort(dst, kind="stable")
    s_src = src[eorder]
    s_dst = dst[eorder]
    starts = np.zeros(NP, dtype=np.int64)
    starts[1:] = np.cumsum(deg)[:-1]
    pos = np.arange(len(s_dst)) - starts[s_dst]
    # pads carry idx=NP: clamped to NP-1 for the gather, masked by the
    # on-device idx>=NP -> -1e30 logit offset
    slot_g = np.full((NP, DMAX), NP, dtype=np.int32)
    slot_g[s_dst, pos] = new_of[s_src].astype(np.int32)
    slot_new = slot_g[glob_of_new]

    deg_sorted = deg[order]
    Dts = tuple(ceil4(max(deg_sorted[1024 * t], 1)) for t in range(TPS))

    att1 = np.asarray(att1, np.float32)
    att2 = np.asarray(att2, np.float32)
    assert np.abs(att1).min() > 1e-8 and np.abs(att2).min() > 1e-8
    p1 = np.argsort(att1 < 0, kind="stable")
    Fp1 = int((att1 >= 0).sum())
    p2 = np.argsort(att2 < 0, kind="stable")
    Fp2 = int((att2 >= 0).sum())
    # fold att into weight columns, sign-permute, and append a row-sum
    # column (the q/sigma channel: sum_f u = x @ wsum)
    def fold(W, att, perm, rowperm=None):
        Wa = (np.asarray(W, np.float32) * att)
        if rowperm is not None:
            Wa = Wa[rowperm, :]
        Wp = Wa[:, perm]
        return np.concatenate([Wp, Wp.sum(1, keepdims=True)], axis=1)

    Wl1a = fold(Wl1, att1, p1)
    Wr1a = fold(Wr1, att1, p1)
    Wl2a = fold(Wl2, att2, p2, rowperm=p1)
    Wr2a = fold(Wr2, att2, p2, rowperm=p1)
    w1c = np.ascontiguousarray(
        np.concatenate([Wl1a, Wr1a], axis=1), np.float16)
    w2c = np.ascontiguousarray(
        np.concatenate([Wl2a, Wr2a], axis=1), np.float16)
    rc1_row = (1.0 / att1[p1]).astype(np.float32)
    rc2_row = (1.0 / att2[p2]).astype(np.float32)
    b1_row = np.asarray(b1, np.float32)[p1]
    b2_row = np.asarray(b2, np.float32)[p2]
    cvec = np.concatenate([rc1_row, b1_row, rc2_row, b2_row]).astype(
        np.float16)

    xp = np.zeros((NP, F_IN), np.float32)
    xp[:N] = x
    xT_perm = xp[glob_of_new].T.astype(np.float16)

    wblob = np.concatenate([w1c.view(np.uint16).ravel(),
                            w2c.view(np.uint16).ravel(),
                            cvec.view(np.uint16).ravel()])
    in_maps = []
    for c in range(NCORES):
        sl = slot_new[ts(c, SH)]
        slp = np.concatenate(
            [sl[ts(t, 128), 0:Dts[t]] for t in range(TPS)],
            axis=1).astype(np.uint16)
        xc = np.ascontiguousarray(xT_perm[:, ts(c, SH)])
        blob = np.concatenate([xc.view(np.uint16).ravel(), wblob,
                               slp.ravel()])
        in_maps.append({"blob": blob})
    return in_maps, Dts, Fp1, Fp2, glob_of_new, p2


def kernel(**inputs):
    global LAST_RESULT, LAST_RUN_WALL
    import time as _time
    in_maps, Dts, Fp1, Fp2, glob_of_new, p2 = prepare_host(**inputs)
    nc = build_program(Dts, Fp1, Fp2)
    _t0 = _time.time()
    res = bass_utils.run_bass_kernel_spmd(nc, in_maps,
                                          core_ids=list(range(NCORES)))
    LAST_RUN_WALL = _time.time() - _t0
    LAST_RESULT = res
    out_new = np.concatenate([res.results[c]["outc"] for c in range(NCORES)],
                             axis=0).astype(np.float32)
    out_glob = np.empty((NP, C), np.float32)
    out_glob[glob_of_new] = out_new
    return np.ascontiguousarray(out_glob[:N][:, np.argsort(p2)])


# revision 34
# speedup vs baseline: 2.5373x; 1.0263x over previous
"""2-layer GATv2 over 50k nodes / 1.6M edges on 8 trn2 NeuronCores.

Strategy (self-contained; shapes hardcoded for this problem):
  - Node-parallel dst sharding: nodes are degree-sorted and dealt round-robin
    to 8 cores (balanced slot counts); each core owns 6272 dst nodes.
  - Per-shard GEMMs only; per-shard xl tables are AllGathered on-device so
    each core can gather arbitrary source rows. Host->device traffic is just
    the core's own x shard (fp16), packed u16 slot tables, small weights and
    one constants row.
  - Per dst node, incoming edges live in up to D=64 "slots" (max degree 61);
    per-128-node-tile slot count Dt comes from the degree sort. Equal-Dt
    tiles are processed in chunks so every edge-phase vector op covers
    several tiles (few, large instructions).
  - att is folded into the weights on the host (u = att*z); leaky-relu
    logits use the identity 0.8*(sum_pos relu(u) - sum_neg relu(-u))
    + 0.2*sigma = 0.6*sigma + 0.4*(sum_pos|u| - sum_neg|u|), computed with
    two abs-reduces; messages are recovered from u via a 1/att columnwise
    multiply (exact up to fp rounding).
  - Gather of xl rows via per-slot indirect DMA (gpsimd SWDGE), bypass mode,
    pad indices clamped to NP-1 and masked to alpha=0 via an on-device
    idx>=NP -> -1e30 logit offset.
  - Layer-1 GEMMs compute [xl|xr] in one matmul pair per tile into a
    persistent SBUF tile; the xl half goes to DRAM in a single DMA for the
    AllGather. Layer-1 output h is transposed on PE into a resident fp16
    SBUF tile used directly as lhsT for the layer-2 GEMMs.
"""
import os
os.environ.setdefault("JAX_PLATFORMS", "cpu")
import sys
if "/opt/trn_rl_repo" not in sys.path:
    sys.path.insert(0, "/opt/trn_rl_repo")
import numpy as np
import concourse.bass as bass
import concourse.bacc as bacc
import concourse.mybir as mybir
import concourse.tile as tile
from concourse import bass_utils
from concourse.masks import make_identity

f32 = mybir.dt.float32
f16 = mybir.dt.float16
i32 = mybir.dt.int32
u16 = mybir.dt.uint16
AX = mybir.AxisListType
OP = mybir.AluOpType
AF = mybir.ActivationFunctionType

N = 50000
NCORES = 8
NP = 50176          # 8 * 6272, multiple of 1024
SH = NP // NCORES   # 6272 = 49 * 128
TPS = SH // 128     # 49 tiles per shard
F_IN = 256
H = 128
C = 64
DMAX = 64
NEG = 0.2
EPS = 1e-16
CAP = 224           # max slots (CH*Dt) per edge-phase chunk

VARIANT = "full"    # "full" | "nogather" | "noedge" (timing ablations)

LAST_RESULT = None
LAST_RUN_WALL = None
_PROGRAM_CACHE = {}


def ts(i, s):
    return slice(i * s, (i + 1) * s)


def ceil4(v):
    return max(4, (int(v) + 3) // 4 * 4)


def make_chunks(Dts, cap=CAP):
    """Group consecutive equal-Dt tiles into chunks of at most cap slots."""
    chunks = []
    t = 0
    while t < TPS:
        Dt = Dts[t]
        ch = 1
        while (t + ch < TPS and Dts[t + ch] == Dt and (ch + 1) * Dt <= cap):
            ch += 1
        chunks.append((t, ch, Dt))
        t += ch
    return chunks


def build_program(Dts, Fp1, Fp2):
    key = (tuple(Dts), Fp1, Fp2, VARIANT)
    if key in _PROGRAM_CACHE:
        return _PROGRAM_CACHE[key]
    TOTD = sum(Dts)
    cums = [0]
    for d in Dts:
        cums.append(cums[-1] + d)
    TW1, TW2 = H + 1, C + 1

    nc = bacc.Bacc("TRN2", target_bir_lowering=False, debug=False,
                   enable_asserts=False, num_devices=NCORES)

    # all inputs packed into one u16 blob (fewer PJRT operands):
    # [xTo f16 | w1 f16 | w2 f16 | cvec f16 | slotp u16]
    LX = F_IN * SH
    LW1 = F_IN * 2 * TW1
    LW2 = H * 2 * TW2
    LCV = 2 * H + 2 * C
    LSL = 128 * TOTD
    o_w1 = LX
    o_w2 = o_w1 + LW1
    o_cv = o_w2 + LW2
    o_sl = o_cv + LCV
    TOTB = o_sl + LSL
    blob = nc.dram_tensor("blob", [TOTB], u16, kind="ExternalInput")
    bap = blob.ap()
    xTo_ap = bap[0:LX].rearrange("(r c) -> r c", r=F_IN).bitcast(f16)
    w1_ap = bap[o_w1:o_w1 + LW1].rearrange("(r c) -> r c",
                                           r=F_IN).bitcast(f16)
    w2_ap = bap[o_w2:o_w2 + LW2].rearrange("(r c) -> r c", r=H).bitcast(f16)
    cv_ap = bap[o_cv:o_cv + LCV].rearrange("(r c) -> r c", r=1).bitcast(f16)
    sl_ap = bap[o_sl:o_sl + LSL].rearrange("(r c) -> r c", r=128)
    outc = nc.dram_tensor("outc", [SH, C], f16, kind="ExternalOutput")

    xl1t = nc.dram_tensor("xl1t", [SH, TW1], f16, kind="Internal")
    xl1f = nc.dram_tensor("xl1f", [NP, TW1], f16, kind="Internal",
                          addr_space="Shared")
    xl2t = nc.dram_tensor("xl2t", [SH, TW2], f16, kind="Internal")
    xl2f = nc.dram_tensor("xl2f", [NP, TW2], f16, kind="Internal",
                          addr_space="Shared")

    with tile.TileContext(nc) as tc:
        with tc.tile_pool(name="pers", bufs=1) as pers:
            # persistent SBUF residents (span all phases)
            x1_sb = pers.tile([128, TPS, 2, TW1], f16)   # [xl|xr] layer 1
            hT_sb = pers.tile([128, SH], f16)
            x2_sb = pers.tile([128, TPS, 2, TW2], f16)   # [xl|xr] layer 2
            slot_i = pers.tile([128, TOTD], i32)
            moff_sb = pers.tile([128, TOTD], f32)
            consts = pers.tile([128, 2 * H + 2 * C], f32)
            ident = pers.tile([128, 128], f32)
            w2_t = pers.tile([128, 2 * TW2], f16)

            make_identity(nc, ident[:])
            nc.sync.dma_start(out=w2_t[:], in_=w2_ap)

            rc1_t = consts[:, 0:H]
            cb1_t = consts[:, H:2 * H]
            rc2_t = consts[:, 2 * H:2 * H + C]
            cb2_t = consts[:, 2 * H + C:2 * H + 2 * C]

            # slot table: u16 load -> f32; mask from idx>=NP; clamp for the
            # gather (pads read row NP-1, masked to alpha=0 by moff)
            with (
                tc.tile_pool(name="pslot", bufs=1) as pslot,
                tc.tile_pool(name="pcp", bufs=1, space="PSUM") as pcp,
            ):
                crow = pslot.tile([1, 2 * H + 2 * C], f16)
                nc.sync.dma_start(out=crow[:], in_=cv_ap)
                ones = pslot.tile([1, 128], f16)
                nc.vector.memset(ones[:], 1.0)
                cps = pcp.tile([128, 2 * H + 2 * C], f32)
                nc.tensor.matmul(out=cps[:], lhsT=ones[:], rhs=crow[:],
                                 start=True, stop=True)
                nc.scalar.copy(out=consts[:], in_=cps[:])

                slot_u = pslot.tile([128, TOTD], u16)
                nc.sync.dma_start(out=slot_u[:], in_=sl_ap)
                slot_f = pslot.tile([128, TOTD], f32)
                nc.vector.tensor_copy(out=slot_f[:], in_=slot_u[:])
                nc.vector.tensor_scalar(out=moff_sb[:], in0=slot_f[:],
                                        scalar1=float(NP) - 0.5,
                                        scalar2=-1e30,
                                        op0=OP.is_ge, op1=OP.mult)
                nc.vector.tensor_scalar_min(slot_f[:], slot_f[:],
                                            float(NP - 1))
                nc.vector.tensor_copy(out=slot_i[:], in_=slot_f[:])

            # ---------------- Phase A: layer-1 GEMMs (own shard) ----------
            with (
                tc.tile_pool(name="paw", bufs=1) as pw,
                tc.tile_pool(name="pap", bufs=4, space="PSUM") as pp,
            ):
                w1_t = pw.tile([128, 2, 2 * TW1], f16)
                for k in range(2):
                    nc.sync.dma_start(out=w1_t[:, k, :],
                                      in_=w1_ap[ts(k, 128), :])
                x_sb = pw.tile([128, 2, SH], f16)
                for k in range(2):
                    nc.sync.dma_start(out=x_sb[:, k, :],
                                      in_=xTo_ap[ts(k, 128), :])
                for t in range(TPS):
                    ps_t = pp.tile([128, 2 * TW1], f32, tag="ps")
                    for k in range(2):
                        nc.tensor.matmul(out=ps_t[:],
                                         lhsT=x_sb[:, k, ts(t, 128)],
                                         rhs=w1_t[:, k, :],
                                         start=(k == 0), stop=(k == 1))
                    nc.scalar.copy(
                        out=x1_sb[:, t, :, :].rearrange("p a w -> p (a w)"),
                        in_=ps_t[:])
                nc.sync.dma_start(
                    out=xl1t.ap().rearrange("(t p) w -> p t w", p=128),
                    in_=x1_sb[:, :, 0, :])

            # ---------------- AllGather layer-1 xl table ------------------
            nc.gpsimd.collective_compute(
                "AllGather", OP.bypass,
                replica_groups=[list(range(NCORES))],
                ins=[xl1t.ap()], outs=[xl1f.ap()])

            # ---------------- layer-1 edge phase --------------------------
            if VARIANT != "noedge":
                edge_phase(nc, tc, Dts, cums, Fp1, H, xl1f, x1_sb, slot_i,
                           moff_sb, rc1_t, cb1_t, relu=True, out_dram=None,
                           hT_sb=hT_sb, ident=ident)
            else:
                nc.vector.memset(hT_sb[:], 0.0)

            # ---------------- Phase C: layer-2 GEMMs (own shard) ----------
            with tc.tile_pool(name="pdp", bufs=4, space="PSUM") as pp2:
                for t in range(TPS):
                    ps2_t = pp2.tile([128, 2 * TW2], f32, tag="ps2")
                    nc.tensor.matmul(out=ps2_t[:], lhsT=hT_sb[:, ts(t, 128)],
                                     rhs=w2_t[:], start=True, stop=True)
                    nc.scalar.copy(
                        out=x2_sb[:, t, :, :].rearrange("p a w -> p (a w)"),
                        in_=ps2_t[:])
                nc.sync.dma_start(
                    out=xl2t.ap().rearrange("(t p) w -> p t w", p=128),
                    in_=x2_sb[:, :, 0, :])

            # ---------------- AllGather layer-2 xl table ------------------
            nc.gpsimd.collective_compute(
                "AllGather", OP.bypass,
                replica_groups=[list(range(NCORES))],
                ins=[xl2t.ap()], outs=[xl2f.ap()])

            # ---------------- layer-2 edge phase --------------------------
            if VARIANT != "noedge":
                edge_phase(nc, tc, Dts, cums, Fp2, C, xl2f, x2_sb, slot_i,
                           moff_sb, rc2_t, cb2_t, relu=False, out_dram=outc,
                           hT_sb=None, ident=None)
            else:
                with tc.tile_pool(name="pdum", bufs=2) as pdum:
                    dt_ = pdum.tile([128, TPS, C], f16)
                    nc.vector.tensor_copy(out=dt_[:], in_=x2_sb[:, :, 1, 0:C])
                    nc.sync.dma_start(
                        out=outc.ap().rearrange("(t p) w -> p t w", p=128),
                        in_=dt_[:])

    nc.compile()
    _PROGRAM_CACHE[key] = nc
    return nc


def edge_phase(nc, tc, Dts, cums, Fp, F, xlf, xlr_sb, slot_i, moff_sb,
               rc_t, cb_t, relu, out_dram, hT_sb, ident):
    TW = F + 1   # table width: F features + q (= row-sum) column
    with (
        tc.tile_pool(name=f"pz{F}", bufs=2) as pz,
        tc.tile_pool(name=f"pm{F}", bufs=3) as psm,
        tc.tile_pool(name=f"po{F}", bufs=2) as pout,
        tc.tile_pool(name=f"pp{F}", bufs=2, space="PSUM") as pps,
    ):
        for (t0, CH, Dt) in make_chunks(Dts):
            S = CH * Dt
            cu = cums[t0]
            off_t = moff_sb[:, cu:cu + S]
            xr_c = xlr_sb[:, t0:t0 + CH, 1, :]

            # z = gathered xl rows (bypass; pad idx clamped to NP-1), then
            # u = z + xr via in-place broadcast add.
            z_t = pz.tile([128, CH, Dt, TW], f16, tag="z")
            zf = z_t[:].rearrange("p c d w -> p (c d) w")
            if VARIANT == "nogather":
                nc.vector.tensor_copy(
                    out=z_t[:],
                    in_=xr_c[:, :, None, :].to_broadcast([128, CH, Dt, TW]))
            else:
                for c in range(CH):
                    cuc = cums[t0 + c]
                    for d in range(Dt):
                        nc.gpsimd.indirect_dma_start(
                            out=z_t[:, c, d, :], out_offset=None,
                            in_=xlf.ap(),
                            in_offset=bass.IndirectOffsetOnAxis(
                                ap=slot_i[:, cuc + d:cuc + d + 1], axis=0),
                            bounds_check=None, compute_op=OP.bypass)
            nc.vector.tensor_tensor(
                out=z_t[:], in0=z_t[:],
                in1=xr_c[:, :, None, :].to_broadcast([128, CH, Dt, TW]),
                op=OP.add)

            # logits. With u = xl+xr (in z) and the q column sigma = sum_f u:
            #   0.8*(sum_pos relu(u) - sum_neg relu(-u)) + 0.2*sigma
            #     = 0.6*sigma + 0.4*(sum_pos |u| - sum_neg |u|)
            # so two abs-reduces replace the ACT relus entirely.
            e_t = psm.tile([128, CH, Dt], f32, tag="e")
            ef = e_t[:].rearrange("p c d -> p (c d)")
            ep_t = psm.tile([128, S], f32, tag="ep")
            nc.vector.tensor_reduce(out=ep_t[:], in_=zf[:, :, 0:Fp],
                                    axis=AX.X, op=OP.add,
                                    apply_absolute_value=True)
            en_t = psm.tile([128, S], f32, tag="en")
            nc.vector.tensor_reduce(out=en_t[:], in_=zf[:, :, Fp:F],
                                    axis=AX.X, op=OP.add,
                                    apply_absolute_value=True)
            nc.vector.scalar_tensor_tensor(out=ef, in0=en_t[:],
                                           scalar=-1.0, in1=ep_t[:],
                                           op0=OP.mult, op1=OP.add)
            nc.vector.scalar_tensor_tensor(out=ef, in0=zf[:, :, F],
                                           scalar=1.5, in1=ef,
                                           op0=OP.mult, op1=OP.add)
            nc.vector.scalar_tensor_tensor(out=ef, in0=ef,
                                           scalar=0.4, in1=off_t,
                                           op0=OP.mult, op1=OP.add)
            # softmax over slots of each dst node (c-row)
            mneg_t = psm.tile([128, CH], f32, tag="mneg")
            nc.vector.tensor_reduce(out=mneg_t[:], in_=e_t[:], axis=AX.X,
                                    op=OP.max, negate=True)
            nc.vector.tensor_scalar_min(mneg_t[:], mneg_t[:], 1e29)
            nc.vector.tensor_tensor(
                out=e_t[:], in0=e_t[:],
                in1=mneg_t[:, :, None].to_broadcast([128, CH, Dt]),
                op=OP.add)
            a_t = psm.tile([128, CH, Dt], f32, tag="a")
            nc.scalar.activation(out=a_t[:].rearrange("p c d -> p (c d)"),
                                 in_=ef, func=AF.Exp)
            s_t = psm.tile([128, CH], f32, tag="s")
            nc.vector.tensor_reduce(out=s_t[:], in_=a_t[:], axis=AX.X,
                                    op=OP.add)
            nc.vector.tensor_scalar_add(s_t[:], s_t[:], EPS)
            r_t = psm.tile([128, CH], f32, tag="r")
            nc.vector.reciprocal(out=r_t[:], in_=s_t[:])
            al_t = psm.tile([128, CH, Dt], f32, tag="al")
            nc.vector.tensor_tensor(
                out=al_t[:], in0=a_t[:],
                in1=r_t[:, :, None].to_broadcast([128, CH, Dt]), op=OP.mult)

            # message aggregation: msg = sum_d alpha_d * u_d - (sum alpha)*xr
            # (z holds u = xr+g). In-place alpha multiply + one reduce over
            # the slot axis via a transposed AP view.
            nc.vector.tensor_tensor(
                out=z_t[:, :, :, 0:F], in0=z_t[:, :, :, 0:F],
                in1=al_t[:, :, :, None].to_broadcast([128, CH, Dt, F]),
                op=OP.mult)
            acc_t = pout.tile([128, CH, F], f32, tag="acc")
            nc.vector.tensor_reduce(
                out=acc_t[:], in_=z_t[:, :, :, 0:F].transpose([0, 1, 3, 2]),
                axis=AX.X, op=OP.add)
            saneg_t = psm.tile([128, CH], f32, tag="saneg")
            nc.vector.tensor_reduce(out=saneg_t[:], in_=al_t[:],
                                    axis=AX.X, op=OP.add, negate=True)
            hh_t = pout.tile([128, CH, F], f32, tag="hh")
            nc.vector.tensor_tensor(
                out=hh_t[:], in0=xr_c[:, :, 0:F],
                in1=saneg_t[:, :, None].to_broadcast([128, CH, F]),
                op=OP.mult)
            nc.vector.tensor_tensor(out=hh_t[:], in0=hh_t[:], in1=acc_t[:],
                                    op=OP.add)
            nc.vector.tensor_tensor(
                out=hh_t[:], in0=hh_t[:],
                in1=rc_t[:, None, :].to_broadcast([128, CH, F]), op=OP.mult)
            if relu:
                nc.vector.tensor_tensor(
                    out=hh_t[:], in0=hh_t[:],
                    in1=cb_t[:, None, :].to_broadcast([128, CH, F]),
                    op=OP.add)
                nc.vector.tensor_scalar_max(hh_t[:], hh_t[:], 0.0)
                for c in range(CH):
                    pt_t = pps.tile([128, 128], f32, tag="pt")
                    nc.tensor.transpose(out=pt_t[:], in_=hh_t[:, c, :],
                                        identity=ident[:])
                    nc.scalar.copy(out=hT_sb[:, ts(t0 + c, 128)], in_=pt_t[:])
            else:
                ho_t = pout.tile([128, CH, F], f16, tag="ho")
                nc.vector.tensor_tensor(
                    out=ho_t[:], in0=hh_t[:],
                    in1=cb_t[:, None, :].to_broadcast([128, CH, F]),
                    op=OP.add)
                nc.sync.dma_start(
                    out=out_dram.ap()[t0 * 128:(t0 + CH) * 128, :].rearrange(
                        "(c p) w -> p c w", p=128),
                    in_=ho_t[:])


def prepare_host(x, edge_index, Wl1, Wr1, att1, b1, Wl2, Wr2, att2, b2):
    src = np.asarray(edge_index[0], dtype=np.int64)
    dst = np.asarray(edge_index[1], dtype=np.int64)
    x = np.asarray(x, dtype=np.float32)

    deg = np.bincount(dst, minlength=NP).astype(np.int64)
    assert deg.max() <= DMAX, f"max degree {deg.max()} > {DMAX}"
    order = np.argsort(-deg, kind="stable")
    q = np.arange(NP)
    new_of = np.empty(NP, dtype=np.int64)
    new_of[order] = (q % NCORES) * SH + q // NCORES
    glob_of_new = np.empty(NP, dtype=np.int64)
    glob_of_new[new_of] = np.arange(NP)

    # slot tables (values are NEW ids; rows ordered by NEW id)
    eorder = np.argsort(dst, kind="stable")
    s_src = src[eorder]
    s_dst = dst[eorder]
    starts = np.zeros(NP, dtype=np.int64)
    starts[1:] = np.cumsum(deg)[:-1]
    pos = np.arange(len(s_dst)) - starts[s_dst]
    # pads carry idx=NP: clamped to NP-1 for the gather, masked by the
    # on-device idx>=NP -> -1e30 logit offset
    slot_g = np.full((NP, DMAX), NP, dtype=np.int32)
    slot_g[s_dst, pos] = new_of[s_src].astype(np.int32)
    slot_new = slot_g[glob_of_new]

    deg_sorted = deg[order]
    Dts = tuple(ceil4(max(deg_sorted[1024 * t], 1)) for t in range(TPS))

    att1 = np.asarray(att1, np.float32)
    att2 = np.asarray(att2, np.float32)
    assert np.abs(att1).min() > 1e-8 and np.abs(att2).min() > 1e-8
    p1 = np.argsort(att1 < 0, kind="stable")
    Fp1 = int((att1 >= 0).sum())
    p2 = np.argsort(att2 < 0, kind="stable")
    Fp2 = int((att2 >= 0).sum())
    # fold att into weight columns, sign-permute, and append a row-sum
    # column (the q/sigma channel: sum_f u = x @ wsum)
    def fold(W, att, perm, rowperm=None):
        Wa = (np.asarray(W, np.float32) * att)
        if rowperm is not None:
            Wa = Wa[rowperm, :]
        Wp = Wa[:, perm]
        return np.concatenate([Wp, Wp.sum(1, keepdims=True)], axis=1)

    Wl1a = fold(Wl1, att1, p1)
    Wr1a = fold(Wr1, att1, p1)
    Wl2a = fold(Wl2, att2, p2, rowperm=p1)
    Wr2a = fold(Wr2, att2, p2, rowperm=p1)
    w1c = np.ascontiguousarray(
        np.concatenate([Wl1a, Wr1a], axis=1), np.float16)
    w2c = np.ascontiguousarray(
        np.concatenate([Wl2a, Wr2a], axis=1), np.float16)
    rc1_row = (1.0 / att1[p1]).astype(np.float32)
    rc2_row = (1.0 / att2[p2]).astype(np.float32)
    b1_row = np.asarray(b1, np.float32)[p1]
    b2_row = np.asarray(b2, np.float32)[p2]
    cvec = np.concatenate([rc1_row, b1_row, rc2_row, b2_row]).astype(
        np.float16)

    xp = np.zeros((NP, F_IN), np.float32)
    xp[:N] = x
    xT_perm = xp[glob_of_new].T.astype(np.float16)

    wblob = np.concatenate([w1c.view(np.uint16).ravel(),
                            w2c.view(np.uint16).ravel(),
                            cvec.view(np.uint16).ravel()])
    in_maps = []
    for c in range(NCORES):
        sl = slot_new[ts(c, SH)]
        slp = np.concatenate(
            [sl[ts(t, 128), 0:Dts[t]] for t in range(TPS)],
            axis=1).astype(np.uint16)
        xc = np.ascontiguousarray(xT_perm[:, ts(c, SH)])
        blob = np.concatenate([xc.view(np.uint16).ravel(), wblob,
                               slp.ravel()])
        in_maps.append({"blob": blob})
    return in_maps, Dts, Fp1, Fp2, glob_of_new, p2


def kernel(**inputs):
    global LAST_RESULT, LAST_RUN_WALL
    import time as _time
    in_maps, Dts, Fp1, Fp2, glob_of_new, p2 = prepare_host(**inputs)
    nc = build_program(Dts, Fp1, Fp2)
    _t0 = _time.time()
    res = bass_utils.run_bass_kernel_spmd(nc, in_maps,
                                          core_ids=list(range(NCORES)))
    LAST_RUN_WALL = _time.time() - _t0
    LAST_RESULT = res
    out_new = np.concatenate([res.results[c]["outc"] for c in range(NCORES)],
                             axis=0).astype(np.float32)
    out_glob = np.empty((NP, C), np.float32)
    out_glob[glob_of_new] = out_new
    return np.ascontiguousarray(out_glob[:N][:, np.argsort(p2)])
